# revision 25
# baseline (speedup 1.0000x reference)
"""Trainium2 Bass kernel for nn_Affinity (gnn_message_passing).

Math (per batch element b, H=128, NP=512, ND=64, DEPTH=3, N=NP*ND=32768):
  drug_feat = lrelu(drug @ d_w + d_b, .1) * du_mask ; u_d = drug_feat.sum(0)
  prot_feat = lrelu(prot @ p_w + p_b, .1) * pu_mask ; u_p = prot_feat.sum(0)
  mij[p,d]  = lrelu(prot_feat[p] @ Wv0a + drug_feat[d] @ Wv0b + Wv0_b, .1)
  m_u       = lrelu([u_d;u_p] @ Wu_w + Wu_b, .01)
  for i in 0..2:   (per pairwise row m)
    t = m @ Wv_i + Wv_b_i ; alpha = m . v_i + c_i    (v_i = Wv_i@att_i,
    g = alpha * t ; m += g                            c_i = Wv_b_i.att_i+att_b_i)
  w1in = m + g   (= m_prev + 2 g_last)
  s = sum_rows relu(w1in @ W1a + W1a_b)
  global1 = s @ W1b + N*W1b_b ; ... small MLP tail ... -> scalar per batch

Strategy: data-parallel over batch, 1 element per NeuronCore, no collectives.
Everything H-major on chip: tensors stored [H=128 partitions, rows free].
The big [N,128] intermediate lives only in SBUF, tiled FD rows at a time.

Engine balance per tile (cost-model ns): PE 9 matmuls (bf16, ~1.9us),
ACT s0-prelu + alpha2 + relu-pair (~1.7us), DVE alpha0 + g0 + two bf16
m-adds (~1.8us), Pool g1 + g2 as scalar_tensor_tensor (~1.6us).
alpha1's +c1 rides the PE as a constant bias-matmul into the pa1 bank.
Main-loop tensors and weights in bf16 (per-row quantization noise washes
out in the final sum over N); prolog/tail stay fp32.
"""

import os
import sys

if "/opt/trn_rl_repo" not in sys.path:
    sys.path.insert(0, "/opt/trn_rl_repo")

import numpy as np
from contextlib import ExitStack

import concourse.bass as bass
import concourse.tile as tile
from concourse import bacc, mybir
from concourse.bass_utils import run_bass_kernel_spmd

F32 = mybir.dt.float32
F32R = mybir.dt.float32r
BF16 = mybir.dt.bfloat16
AF = mybir.ActivationFunctionType
OP = mybir.AluOpType

B, NP, ND, H, DEPTH = 8, 512, 64, 128, 3
N = NP * ND
FD = 512           # rows per macro tile (TD=FD//NP d-values per tile)
TD = FD // NP      # d-values per macro tile
NT = N // FD       # macro tiles per core

# main-loop weights, shipped as their own tensor (used as F32R lhsT in-place)
WM_NAMES = [
    "Wv_0", "Wv_1", "Wv_2", "vrep_0", "vrep_1", "vrep_2", "W1a",
]
WMIDX = {n: i for i, n in enumerate(WM_NAMES)}
NWM = len(WM_NAMES)

# prolog/tail weight slot order in the packed [128, 128*NW] weight matrix
W_NAMES = [
    "p_w", "d_w", "Wv0a", "Wv0b",
    "W1b", "W2a_0", "W2a_1", "W2b_0", "W2b_1", "W3",
    "Wu_0", "Wu_1", "Wu1a_0", "Wu1a_1", "Wu1b",
]
WIDX = {n: i for i, n in enumerate(W_NAMES)}
NW = len(W_NAMES)

B_NAMES = [
    "p_b", "d_b", "Wv0_b", "Wv_b0", "Wv_b1", "Wv_b2", "c0", "c1", "c2x2",
    "Wu_b", "W1a_b", "W1b_bN", "W2a_b0", "W2a_b1", "W2b_b", "W3_b",
    "Wu1a_b", "Wu1b_b", "W5_b", "W5_w",
]
BIDX = {n: i for i, n in enumerate(B_NAMES)}
NB = len(B_NAMES)


_CONSTS = None    # (c0, c1, 2*c2) baked as immediates; set by _host_prep


def build_program():
    import os as _os
    c0_v, c1_v, c2x2_v = _CONSTS if _CONSTS is not None else (0.01, 0.01, 0.02)
    nc = bacc.Bacc("TRN2", target_bir_lowering=False, debug=False, num_devices=8)

    pfT_d = nc.dram_tensor("pfT", [H, NP], F32, kind="ExternalInput")
    dfT_d = nc.dram_tensor("dfT", [H, ND], F32, kind="ExternalInput")
    pmask_d = nc.dram_tensor("pmask", [H, NP], F32, kind="ExternalInput")
    dmask_d = nc.dram_tensor("dmask", [H, ND], F32, kind="ExternalInput")
    wmat_d = nc.dram_tensor("wmat", [H, 128 * NW], F32, kind="ExternalInput")
    wmain_d = nc.dram_tensor("wmain", [H, 128 * NWM], BF16, kind="ExternalInput")
    bmat_d = nc.dram_tensor("bmat", [H, NB], F32, kind="ExternalInput")
    out_d = nc.dram_tensor("out", [1, 1], F32, kind="ExternalOutput")

    def w(ap, name):
        i = WIDX[name]
        return ap[:, i * 128:(i + 1) * 128]

    def bcol(ap, name):
        i = BIDX[name]
        return ap[:, i:i + 1]

    with tile.TileContext(nc) as tc, ExitStack() as ctx:
        sbw = ctx.enter_context(tc.tile_pool(name="sbw", bufs=1))

        # prefetch the ACT table set (Prelu/Identity/Relu live in one set)
        # concurrently with the input DMAs instead of stalling the first tile
        warm = sbw.tile([1, 1], F32, tag="warm")
        nc.vector.memset(warm[:], 0.0)
        nc.scalar.activation(warm[:], warm[:], AF.Prelu, bias=0.0, scale=1.0,
                             alpha=0.1)
        wjunk = sbw.tile([H, FD], BF16, tag="wjunk")
        nc.gpsimd.memset(wjunk[:], 0.0)
        with tc.tile_pool(name="pswarm", bufs=1, space="PSUM") as pswarm:
            pjunk = pswarm.tile([H, FD], F32, tag="pjunk")
            for _ in range(12):
                nc.tensor.matmul(pjunk[:], wjunk[:, 0:128],
                                 wjunk[:], start=True, stop=True)

        # startup DMAs spread across engine queues so the HWDGEs run in
        # parallel instead of serializing on the SP queue
        wmain = sbw.tile([H, 128 * NWM], BF16, tag="wmain")
        nc.sync.dma_start(out=wmain[:], in_=wmain_d[:])
        wsb = sbw.tile([H, 128 * NW], F32, tag="wsb")
        qs = [nc.scalar, nc.vector, nc.gpsimd, nc.sync]
        for qi, c0 in enumerate(range(0, 128 * NW, 128 * 4)):
            c1 = min(c0 + 128 * 4, 128 * NW)
            qs[qi % 4].dma_start(out=wsb[:, c0:c1], in_=wmat_d[:, c0:c1])
        bsb = sbw.tile([H, NB], F32, tag="bsb")
        nc.sync.dma_start(out=bsb[:], in_=bmat_d[:])
        pfT = sbw.tile([H, NP], F32, tag="pfT")
        nc.scalar.dma_start(out=pfT[:], in_=pfT_d[:])
        dfT = sbw.tile([H, ND], F32, tag="dfT")
        nc.gpsimd.dma_start(out=dfT[:], in_=dfT_d[:])
        pmask = sbw.tile([H, NP], F32, tag="pmask")
        nc.vector.dma_start(out=pmask[:], in_=pmask_d[:])
        dmask = sbw.tile([H, ND], F32, tag="dmask")
        nc.vector.dma_start(out=dmask[:], in_=dmask_d[:])

        # main-loop weights used in place as BF16 lhsT; ones rhs for bias-MMs
        wr = {n: wmain[:, i * 128:(i + 1) * 128] for n, i in WMIDX.items()}
        ones = sbw.tile([H, FD], BF16, tag="ones")
        nc.vector.memset(ones[:], 1.0)
        # rounded F32R copies for the prolog GEMMs (DMA output may not feed
        # an FP32r matmult directly -- the producer must round)
        pr = {}
        for name in ["p_w", "d_w", "Wv0a", "Wv0b"]:
            t = sbw.tile([H, 128], F32R, tag=f"p_{name}")
            nc.vector.tensor_copy(t[:], w(wsb, name))
            pr[name] = t
        pfT_r = sbw.tile([H, NP], F32R, tag="pfT_r")
        nc.vector.tensor_copy(pfT_r[:], pfT[:])
        dfT_r = sbw.tile([H, ND], F32R, tag="dfT_r")
        nc.vector.tensor_copy(dfT_r[:], dfT[:])

        # ---- startup: projections, u_d/u_p, m_u, prot/drug proj ----
        ppT = sbw.tile([H, NP], F32, tag="ppT")
        dpT = sbw.tile([H, ND], F32, tag="dpT")
        u_p = sbw.tile([H, 1], F32, tag="u_p")
        u_d = sbw.tile([H, 1], F32, tag="u_d")
        mu_t = sbw.tile([H, 1], F32, tag="mu_t")
        acc = sbw.tile([H, NT], F32, tag="acc")

        with tc.tile_pool(name="ps0", bufs=2, space="PSUM") as ps0:
            # prot: feat = prelu(p_w.T @ pfT + p_b); masked + u_p
            ps_pf = ps0.tile([H, NP], F32, tag="ps0")
            nc.tensor.matmul(ps_pf[:], pr["p_w"][:], pfT_r[:], start=True, stop=True)
            pf_act = sbw.tile([H, NP], F32, tag="pf_act")
            nc.scalar.activation(pf_act[:], ps_pf[:], AF.Prelu,
                                 bias=bcol(bsb, "p_b"), scale=1.0, alpha=0.1)
            pf_m = sbw.tile([H, NP], F32R, tag="pf_m")
            nc.vector.scalar_tensor_tensor(pf_m[:], pf_act[:], 1.0, pmask[:],
                                           op0=OP.mult, op1=OP.mult,
                                           accum_out=u_p[:])
            # drug
            ps_df = ps0.tile([H, ND], F32, tag="ps0")
            nc.tensor.matmul(ps_df[:], pr["d_w"][:], dfT_r[:], start=True, stop=True)
            df_act = sbw.tile([H, ND], F32, tag="df_act")
            nc.scalar.activation(df_act[:], ps_df[:], AF.Prelu,
                                 bias=bcol(bsb, "d_b"), scale=1.0, alpha=0.1)
            df_m = sbw.tile([H, ND], F32R, tag="df_m")
            nc.vector.scalar_tensor_tensor(df_m[:], df_act[:], 1.0, dmask[:],
                                           op0=OP.mult, op1=OP.mult,
                                           accum_out=u_d[:])
            # pairwise projections (Wv0_b folded into prot side)
            ps_pp = ps0.tile([H, NP], F32, tag="ps0")
            nc.tensor.matmul(ps_pp[:], pr["Wv0a"][:], pf_m[:], start=True, stop=True)
            nc.scalar.activation(ppT[:], ps_pp[:], AF.Identity,
                                 bias=bcol(bsb, "Wv0_b"), scale=1.0)
            ps_dp = ps0.tile([H, ND], F32, tag="ps0")
            nc.tensor.matmul(ps_dp[:], pr["Wv0b"][:], df_m[:], start=True, stop=True)
            nc.scalar.copy(dpT[:], ps_dp[:])
            # m_u = prelu(Wu.T @ [u_d;u_p] + Wu_b, .01)
            ps_mu = ps0.tile([H, 1], F32, tag="ps0s")
            nc.tensor.matmul(ps_mu[:], w(wsb, "Wu_0"), u_d[:], start=True, stop=False)
            nc.tensor.matmul(ps_mu[:], w(wsb, "Wu_1"), u_p[:], start=False, stop=True)
            nc.scalar.activation(mu_t[:], ps_mu[:], AF.Prelu,
                                 bias=bcol(bsb, "Wu_b"), scale=1.0, alpha=0.01)

        # ---- main loop over macro tiles (software-pipelined emission) ----
        # Stages per tile t:
        #   S0: construct m0        S1/S2/S3: depth 0/1/2        S4: W1a+relu
        # Emitted deepest-stage-first per step so each engine's instruction
        # stream interleaves stages of staggered tiles (no cross-tile stalls).
        PSA = int(_os.environ.get("K_PSA", "3"))
        PST = int(_os.environ.get("K_PST", "3"))
        PSW = int(_os.environ.get("K_PSW", "1"))
        PAIR = int(_os.environ.get("K_PAIR", "1"))
        with tc.tile_pool(name="sbm", bufs=16) as sbm, \
             tc.tile_pool(name="sbg", bufs=8) as sbg, \
             tc.tile_pool(name="pst", bufs=PST, space="PSUM") as pst, \
             tc.tile_pool(name="psa", bufs=PSA, space="PSUM") as psa, \
             tc.tile_pool(name="psw", bufs=PSW, space="PSUM") as psw:
            NTL = NT
            # test.py wraps the main loop in an on-device For_i for timing
            reps = int(_os.environ.get("K_REPS", "0"))
            m0set, m1set, m2set, g_set = {}, {}, {}, {}
            pa_set, pt_set, al_set, pwpair = {}, {}, {}, {}

            # Legal-ISA schedule (GPSIMD cannot touch PSUM; every other op
            # reads at most ONE PSUM operand, so each depth needs an
            # alpha-mover (ACT/DVE) plus a g = (pt+b)*al stt on DVE):
            #   al_i = pa_i + c_i   (PSUM->SBUF, ACT Identity-bias or DVE ts)
            #   g_i  = (pt_i + b_i) * al_i    [DVE stt, pt in PSUM]
            #   m   += g                      [Pool tt, SBUF bf16]
            # vrep_2/c2 are pre-doubled so g2 = 2*g2_true and the W1a stage
            # is W1a@m2 + W1a@g2 accumulated in PSUM (w1in never built).
            # ACT carries ~2.14 of the 3 movers (s0 + relu fill the rest);
            # the remainder runs on DVE, mixed by tile parity.

            AL2ACT = int(_os.environ.get("K_AL2ACT", "7"))  # al2 on ACT every Nth
            AL1DVE = int(_os.environ.get("K_AL1DVE", "0"))  # al1 on DVE every Nth
            ADDDVE = int(_os.environ.get("K_ADDDVE", "0"))  # m-adds on DVE every Nth

            def d_mm(i, t, m):
                pa = psa.tile([H, FD], F32, tag="pa")
                pt = pst.tile([H, FD], F32, tag="pt")
                nc.tensor.matmul(pa[:], wr[f"vrep_{i}"], m[:],
                                 start=True, stop=True)
                nc.tensor.matmul(pt[:], wr[f"Wv_{i}"], m[:],
                                 start=True, stop=True)
                pa_set[(i, t)], pt_set[(i, t)] = pa, pt

            def d_al(i, t, on_act):
                pa = pa_set.pop((i, t))
                al = sbg.tile([H, FD], BF16, tag=f"al{i}")
                cname = ("c0", "c1", "c2x2")[i]
                if on_act:
                    nc.scalar.activation(al[:], pa[:], AF.Identity,
                                         bias=bcol(bsb, cname), scale=1.0)
                else:
                    nc.vector.tensor_scalar_add(al[:], pa[:],
                                                float((c0_v, c1_v, c2x2_v)[i]))
                al_set[(i, t)] = al

            def d_g(i, t):
                pt, al = pt_set.pop((i, t)), al_set.pop((i, t))
                g = sbg.tile([H, FD], BF16, tag=f"g{i}")
                nc.vector.scalar_tensor_tensor(g[:], pt[:],
                                               bcol(bsb, f"Wv_b{i}"), al[:],
                                               op0=OP.add, op1=OP.mult)
                g_set[(i, t)] = g

            def d_add(i, t, msrc, mdst_set):
                g = g_set.pop((i, t))
                m2 = sbm.tile([H, FD], BF16, tag=f"m{i + 1}")
                if ADDDVE and t % ADDDVE == 0:
                    nc.vector.tensor_tensor(m2[:], msrc.pop(t)[:], g[:],
                                            op=OP.add)
                else:
                    nc.gpsimd.tensor_tensor(m2[:], msrc.pop(t)[:], g[:],
                                            op=OP.add)
                mdst_set[t] = m2

            def s0(t):
                m0 = sbm.tile([H, FD], BF16, tag="m0")
                nc.scalar.activation(m0[:], ppT[:], AF.Prelu,
                                     bias=dpT[:, t:t + 1], scale=1.0, alpha=0.1)
                m0set[t] = m0

            def s4_mm(t):
                m2, g2 = m2set.pop(t), g_set.pop((2, t))
                if PAIR:
                    if t % 2 == 0:
                        pwpair[0] = psw.tile([H, 2 * FD], F32, tag="pw",
                                             name="pwp")
                    pw = pwpair[0]
                    half = slice((t % 2) * FD, (t % 2 + 1) * FD)
                else:
                    pwpair[0] = psw.tile([H, FD], F32, tag="pw", name="pwp")
                    pw = pwpair[0]
                    half = slice(0, FD)
                nc.tensor.matmul(pw[:, half], wr["W1a"], m2[:],
                                 start=True, stop=False)
                nc.tensor.matmul(pw[:, half], wr["W1a"], g2[:],
                                 start=False, stop=True)

            def s4_relu(t):
                if PAIR:
                    if t % 2 == 1:
                        pw = pwpair[0]
                        scr = sbg.tile([H, 2 * FD], BF16, tag="scr")
                        nc.scalar.activation(scr[:], pw[:], AF.Relu,
                                             bias=bcol(bsb, "W1a_b"), scale=1.0,
                                             accum_out=acc[:, t // 2:t // 2 + 1])
                else:
                    pw = pwpair[0]
                    scr = sbg.tile([H, FD], BF16, tag="scr")
                    nc.scalar.activation(scr[:], pw[:], AF.Relu,
                                         bias=bcol(bsb, "W1a_b"), scale=1.0,
                                         accum_out=acc[:, t:t + 1])

            LAG = int(_os.environ.get('K_LAG', '3'))

            def main_loop():
                for step in range(NTL + 4 * LAG):
                    t1 = step - LAG          # depth-0 tile
                    t2 = step - 2 * LAG      # depth-1 tile
                    t3 = step - 3 * LAG      # depth-2 tile
                    t4 = step - 4 * LAG      # W1a tile
                    live1 = 0 <= t1 < NTL
                    live2 = 0 <= t2 < NTL
                    live3 = 0 <= t3 < NTL
                    live4 = 0 <= t4 < NTL
                    al2_act = AL2ACT and t3 % AL2ACT == 0
                    al1_dve = AL1DVE and t2 % AL1DVE == 0
                    if live1:
                        d_mm(0, t1, m0set[t1])
                        d_al(0, t1, True)       # ACT
                        d_g(0, t1)              # DVE
                    if live3:
                        d_mm(2, t3, m2set[t3])
                        d_al(2, t3, al2_act)    # DVE (ACT every Nth)
                    if step < NTL:
                        s0(step)
                    if live3:
                        d_g(2, t3)              # DVE
                    if live1:
                        d_add(0, t1, m0set, m1set)   # Pool
                    if live2:
                        d_mm(1, t2, m1set[t2])
                        d_al(1, t2, not al1_dve)     # ACT (DVE every Nth)
                        d_g(1, t2)              # DVE
                        d_add(1, t2, m1set, m2set)   # Pool
                    if live4:
                        s4_mm(t4)
                        s4_relu(t4)

            if reps:
                with tc.For_i(0, reps, 1,
                              hint_engines=(mybir.EngineType.PE,
                                            mybir.EngineType.Activation,
                                            mybir.EngineType.DVE,
                                            mybir.EngineType.Pool,
                                            mybir.EngineType.SP)):
                    main_loop()
            else:
                main_loop()


        # ---- tail MLP (tiny) ----
        with tc.tile_pool(name="pse", bufs=2, space="PSUM") as pse, \
             tc.tile_pool(name="sbe", bufs=1) as sbe:
            s_t = sbe.tile([H, 1], F32, tag="s")
            nacc = NT // 2 if int(_os.environ.get("K_PAIR", "1")) else NT
            nc.vector.tensor_reduce(s_t[:], acc[:, 0:nacc],
                                    axis=mybir.AxisListType.X, op=OP.add)
            p1 = pse.tile([H, 1], F32, tag="pse")
            nc.tensor.matmul(p1[:], w(wsb, "W1b"), s_t[:], start=True, stop=True)
            t1 = sbe.tile([H, 1], F32, tag="t1")
            nc.scalar.activation(t1[:], p1[:], AF.Identity,
                                 bias=bcol(bsb, "W1b_bN"), scale=1.0)
            p2a = pse.tile([H, 2], F32, tag="pse")
            nc.tensor.matmul(p2a[:, 0:1], w(wsb, "W2a_0"), t1[:], start=True, stop=True)
            nc.tensor.matmul(p2a[:, 1:2], w(wsb, "W2a_1"), t1[:], start=True, stop=True)
            t2a = sbe.tile([H, 2], F32, tag="t2a")
            nc.scalar.activation(t2a[:, 0:1], p2a[:, 0:1], AF.Prelu,
                                 bias=bcol(bsb, "W2a_b0"), scale=1.0, alpha=0.1)
            nc.scalar.activation(t2a[:, 1:2], p2a[:, 1:2], AF.Prelu,
                                 bias=bcol(bsb, "W2a_b1"), scale=1.0, alpha=0.1)
            p2 = pse.tile([H, 1], F32, tag="pse")
            nc.tensor.matmul(p2[:], w(wsb, "W2b_0"), t2a[:, 0:1], start=True, stop=False)
            nc.tensor.matmul(p2[:], w(wsb, "W2b_1"), t2a[:, 1:2], start=False, stop=True)
            t2 = sbe.tile([H, 1], F32, tag="t2")
            nc.scalar.activation(t2[:], p2[:], AF.Identity,
                                 bias=bcol(bsb, "W2b_b"), scale=1.0)
            p3 = pse.tile([H, 1], F32, tag="pse")
            nc.tensor.matmul(p3[:], w(wsb, "W3"), t2[:], start=True, stop=True)
            t3 = sbe.tile([H, 1], F32, tag="t3")
            nc.scalar.activation(t3[:], p3[:], AF.Prelu,
                                 bias=bcol(bsb, "W3_b"), scale=1.0, alpha=0.1)
            p4 = pse.tile([H, 1], F32, tag="pse")
            nc.tensor.matmul(p4[:], w(wsb, "Wu1a_0"), mu_t[:], start=True, stop=False)
            nc.tensor.matmul(p4[:], w(wsb, "Wu1a_1"), t3[:], start=False, stop=True)
            t4 = sbe.tile([H, 1], F32, tag="t4")
            nc.scalar.activation(t4[:], p4[:], AF.Prelu,
                                 bias=bcol(bsb, "Wu1a_b"), scale=1.0, alpha=0.1)
            p5 = pse.tile([H, 1], F32, tag="pse")
            nc.tensor.matmul(p5[:], w(wsb, "Wu1b"), t4[:], start=True, stop=True)
            t5 = sbe.tile([H, 1], F32, tag="t5")
            nc.scalar.activation(t5[:], p5[:], AF.Identity,
                                 bias=bcol(bsb, "Wu1b_b"), scale=1.0)
            p6 = pse.tile([1, 1], F32, tag="pse")
            nc.tensor.matmul(p6[:], bcol(bsb, "W5_w"), t5[:], start=True, stop=True)
            o_sb = sbe.tile([1, 1], F32, tag="o")
            nc.scalar.activation(o_sb[:], p6[:], AF.Identity,
                                 bias=bsb[0:1, BIDX["W5_b"]:BIDX["W5_b"] + 1],
                                 scale=1.0)
            nc.sync.dma_start(out=out_d[:], in_=o_sb[:])

    nc.compile()
    return nc


_NC = None


def _get_nc():
    global _NC
    if _NC is None or _NC[0] != _CONSTS:
        _NC = (_CONSTS, build_program())
    return _NC[1]


def _host_prep(inputs):
    """Build per-core in_maps from full inputs (weight transforms on host)."""
    f = {k: np.asarray(v, dtype=np.float32) for k, v in inputs.items()}

    # v_i = Wv_w[i] @ att_w[i],  c_i = att_w[i].Wv_b[i] + att_b[i]
    Wv_w, att_w = f["Wv_w"], f["att_w"]
    Wv_b, att_b = f["Wv_b"], f["att_b"]
    vs = [Wv_w[i] @ att_w[i] for i in range(DEPTH)]
    cs = [float(att_w[i] @ Wv_b[i] + att_b[i]) for i in range(DEPTH)]
    global _CONSTS
    _CONSTS = (cs[0], cs[1], 2.0 * cs[2])

    wmcols = {
        "Wv_0": Wv_w[0], "Wv_1": Wv_w[1], "Wv_2": Wv_w[2],
        "vrep_0": np.repeat(vs[0][:, None], 128, 1),
        "vrep_1": np.repeat(vs[1][:, None], 128, 1),
        "vrep_2": np.repeat(2.0 * vs[2][:, None], 128, 1),
        "W1a": f["W1a_w"],
    }
    import ml_dtypes
    wmain = np.concatenate([np.ascontiguousarray(wmcols[n])
                            for n in WM_NAMES], axis=1).astype(ml_dtypes.bfloat16)
    wcols = {
        "p_w": f["p_w"], "d_w": f["d_w"],
        "Wv0a": f["Wv0_w"][:H], "Wv0b": f["Wv0_w"][H:],
        "W1b": f["W1b_w"],
        "W2a_0": f["W2a_w"][:, :H], "W2a_1": f["W2a_w"][:, H:],
        "W2b_0": f["W2b_w"][:H], "W2b_1": f["W2b_w"][H:],
        "W3": f["W3_w"],
        "Wu_0": f["Wu_w"][:H], "Wu_1": f["Wu_w"][H:],
        "Wu1a_0": f["Wu1a_w"][:H], "Wu1a_1": f["Wu1a_w"][H:],
        "Wu1b": f["Wu1b_w"],
    }
    wmat = np.concatenate([np.ascontiguousarray(wcols[n]) for n in W_NAMES],
                          axis=1)

    def bc(v):
        v = np.asarray(v, dtype=np.float32).reshape(-1)
        if v.size == 1:
            return np.full((H,), float(v[0]), dtype=np.float32)
        assert v.size == H
        return v

    bvals = {
        "p_b": f["p_b"], "d_b": f["d_b"], "Wv0_b": f["Wv0_b"],
        "Wv_b0": Wv_b[0], "Wv_b1": Wv_b[1], "Wv_b2": Wv_b[2],
        "c0": cs[0], "c1": cs[1], "c2x2": 2.0 * cs[2],
        "Wu_b": f["Wu_b"], "W1a_b": f["W1a_b"],
        "W1b_bN": f["W1b_b"] * np.float32(N),
        "W2a_b0": f["W2a_b"][:H], "W2a_b1": f["W2a_b"][H:],
        "W2b_b": f["W2b_b"], "W3_b": f["W3_b"],
        "Wu1a_b": f["Wu1a_b"], "Wu1b_b": f["Wu1b_b"],
        "W5_b": f["W5_b"], "W5_w": f["W5_w"][:, 0],
    }
    bmat = np.stack([bc(bvals[n]) for n in B_NAMES], axis=1)
    bmat = np.ascontiguousarray(bmat, dtype=np.float32)

    pf, df = f["protein_features"], f["drug_features"]
    pm, dm = f["pu_mask"], f["du_mask"]
    in_maps = []
    for b in range(B):
        in_maps.append({
            "pfT": np.ascontiguousarray(pf[b].T),
            "dfT": np.ascontiguousarray(df[b].T),
            "pmask": np.ascontiguousarray(
                np.broadcast_to(pm[b][None, :], (H, NP))),
            "dmask": np.ascontiguousarray(
                np.broadcast_to(dm[b][None, :], (H, ND))),
            "wmat": wmat,
            "wmain": wmain,
            "bmat": bmat,
        })
    return in_maps


def kernel(**inputs) -> np.ndarray:
    in_maps = _host_prep(inputs)
    nc = _get_nc()
    res = run_bass_kernel_spmd(nc, in_maps, list(range(B)))
    out = np.concatenate([res.results[b]["out"] for b in range(B)], axis=0)
    return out.astype(np.float32).reshape(B, 1)


# revision 28
# speedup vs baseline: 1.1842x; 1.1842x over previous
"""Trainium2 Bass kernel for nn_Affinity (gnn_message_passing).

Math (per batch element b, H=128, NP=512, ND=64, DEPTH=3, N=NP*ND=32768):
  drug_feat = lrelu(drug @ d_w + d_b, .1) * du_mask ; u_d = drug_feat.sum(0)
  prot_feat = lrelu(prot @ p_w + p_b, .1) * pu_mask ; u_p = prot_feat.sum(0)
  mij[p,d]  = lrelu(prot_feat[p] @ Wv0a + drug_feat[d] @ Wv0b + Wv0_b, .1)
  m_u       = lrelu([u_d;u_p] @ Wu_w + Wu_b, .01)
  for i in 0..2:   (per pairwise row m)
    t = m @ Wv_i + Wv_b_i ; alpha = m . v_i + c_i    (v_i = Wv_i@att_i,
    g = alpha * t ; m += g                            c_i = Wv_b_i.att_i+att_b_i)
  w1in = m + g   (= m_prev + 2 g_last)
  s = sum_rows relu(w1in @ W1a + W1a_b)
  global1 = s @ W1b + N*W1b_b ; ... small MLP tail ... -> scalar per batch

Strategy: data-parallel over batch, 1 element per NeuronCore, no collectives.
Everything H-major on chip: tensors stored [H=128 partitions, rows free].
The big [N,128] intermediate lives only in SBUF, tiled FD rows at a time.

Engine balance per tile (cost-model ns): PE 9 matmuls (bf16, ~1.9us),
ACT s0-prelu + alpha2 + relu-pair (~1.7us), DVE alpha0 + g0 + two bf16
m-adds (~1.8us), Pool g1 + g2 as scalar_tensor_tensor (~1.6us).
alpha1's +c1 rides the PE as a constant bias-matmul into the pa1 bank.
Main-loop tensors and weights in bf16 (per-row quantization noise washes
out in the final sum over N); prolog/tail stay fp32.
"""

import os
import sys

if "/opt/trn_rl_repo" not in sys.path:
    sys.path.insert(0, "/opt/trn_rl_repo")

import numpy as np
from contextlib import ExitStack

import concourse.bass as bass
import concourse.tile as tile
from concourse import bacc, mybir
from concourse.bass_utils import run_bass_kernel_spmd

F32 = mybir.dt.float32
F32R = mybir.dt.float32r
BF16 = mybir.dt.bfloat16
AF = mybir.ActivationFunctionType
OP = mybir.AluOpType

B, NP, ND, H, DEPTH = 8, 512, 64, 128, 3
N = NP * ND
FD = 512           # rows per macro tile (TD=FD//NP d-values per tile)
TD = FD // NP      # d-values per macro tile
NT = N // FD       # macro tiles per core

# main-loop weights, shipped as their own tensor (used as F32R lhsT in-place)
WM_NAMES = [
    "Wv_0", "Wv_1", "Wv_2", "vrep_0", "vrep_1", "vrep_2", "W1a",
]
WMIDX = {n: i for i, n in enumerate(WM_NAMES)}
NWM = len(WM_NAMES)

# prolog/tail weight slot order in the packed [128, 128*NW] weight matrix
W_NAMES = [
    "p_w", "d_w", "Wv0a", "Wv0b",
    "W1b", "W2a_0", "W2a_1", "W2b_0", "W2b_1", "W3",
    "Wu_0", "Wu_1", "Wu1a_0", "Wu1a_1", "Wu1b",
]
WIDX = {n: i for i, n in enumerate(W_NAMES)}
NW = len(W_NAMES)

B_NAMES = [
    "p_b", "d_b", "Wv0_b", "Wv_b0", "Wv_b1", "Wv_b2", "c0", "c1", "c2x2",
    "Wu_b", "W1a_b", "W1b_bN", "W2a_b0", "W2a_b1", "W2b_b", "W3_b",
    "Wu1a_b", "Wu1b_b", "W5_b", "W5_w",
]
BIDX = {n: i for i, n in enumerate(B_NAMES)}
NB = len(B_NAMES)


_CONSTS = None    # (c0, c1, 2*c2) baked as immediates; set by _host_prep


def build_program():
    import os as _os
    c0_v, c1_v, c2x2_v = _CONSTS if _CONSTS is not None else (0.01, 0.01, 0.02)
    F32R_MODE = _os.environ.get("K_DT", "bf16") == "f32r"
    MD = F32R if F32R_MODE else BF16
    nc = bacc.Bacc("TRN2", target_bir_lowering=False, debug=False, num_devices=8)

    pfT_d = nc.dram_tensor("pfT", [H, NP], F32, kind="ExternalInput")
    dfT_d = nc.dram_tensor("dfT", [H, ND], F32, kind="ExternalInput")
    pmask_d = nc.dram_tensor("pmask", [H, NP], F32, kind="ExternalInput")
    dmask_d = nc.dram_tensor("dmask", [H, ND], F32, kind="ExternalInput")
    wmat_d = nc.dram_tensor("wmat", [H, 128 * NW], F32, kind="ExternalInput")
    wmain_d = nc.dram_tensor("wmain", [H, 128 * NWM],
                             F32 if F32R_MODE else BF16,
                             kind="ExternalInput")
    bmat_d = nc.dram_tensor("bmat", [H, NB], F32, kind="ExternalInput")
    out_d = nc.dram_tensor("out", [1, 1], F32, kind="ExternalOutput")

    def w(ap, name):
        i = WIDX[name]
        return ap[:, i * 128:(i + 1) * 128]

    def bcol(ap, name):
        i = BIDX[name]
        return ap[:, i:i + 1]

    with tile.TileContext(nc) as tc, ExitStack() as ctx:
        sbw = ctx.enter_context(tc.tile_pool(name="sbw", bufs=1))

        # prefetch the ACT table set (Prelu/Identity/Relu live in one set)
        # concurrently with the input DMAs instead of stalling the first tile
        warm = sbw.tile([1, 1], F32, tag="warm")
        nc.vector.memset(warm[:], 0.0)
        nc.scalar.activation(warm[:], warm[:], AF.Prelu, bias=0.0, scale=1.0,
                             alpha=0.1)
        wjunk = sbw.tile([H, FD], BF16, tag="wjunk")
        nc.gpsimd.memset(wjunk[:], 0.0)
        with tc.tile_pool(name="pswarm", bufs=1, space="PSUM") as pswarm:
            pjunk = pswarm.tile([H, FD], F32, tag="pjunk")
            for _ in range(12):
                nc.tensor.matmul(pjunk[:], wjunk[:, 0:128],
                                 wjunk[:], start=True, stop=True)

        # startup DMAs spread across engine queues so the HWDGEs run in
        # parallel instead of serializing on the SP queue
        wmain = sbw.tile([H, 128 * NWM], F32 if F32R_MODE else BF16,
                         tag="wmain")
        nc.sync.dma_start(out=wmain[:], in_=wmain_d[:])
        wsb = sbw.tile([H, 128 * NW], F32, tag="wsb")
        qs = [nc.scalar, nc.sync]
        for qi, c0 in enumerate(range(0, 128 * NW, 128 * 4)):
            c1 = min(c0 + 128 * 4, 128 * NW)
            qs[qi % 2].dma_start(out=wsb[:, c0:c1], in_=wmat_d[:, c0:c1])
        bsb = sbw.tile([H, NB], F32, tag="bsb")
        nc.sync.dma_start(out=bsb[:], in_=bmat_d[:])
        pfT = sbw.tile([H, NP], F32, tag="pfT")
        nc.scalar.dma_start(out=pfT[:], in_=pfT_d[:])
        dfT = sbw.tile([H, ND], F32, tag="dfT")
        nc.scalar.dma_start(out=dfT[:], in_=dfT_d[:])
        pmask = sbw.tile([H, NP], F32, tag="pmask")
        nc.scalar.dma_start(out=pmask[:], in_=pmask_d[:])
        dmask = sbw.tile([H, ND], F32, tag="dmask")
        nc.scalar.dma_start(out=dmask[:], in_=dmask_d[:])

        # main-loop weights: BF16 used in place; F32R via rounded copies
        if F32R_MODE:
            wr = {}
            for n, i in WMIDX.items():
                t = sbw.tile([H, 128], F32R, tag=f"r_{n}")
                nc.vector.tensor_copy(t[:], wmain[:, i * 128:(i + 1) * 128])
                wr[n] = t[:]
        else:
            wr = {n: wmain[:, i * 128:(i + 1) * 128] for n, i in WMIDX.items()}
        ones = sbw.tile([H, FD], BF16, tag="ones")
        nc.vector.memset(ones[:], 1.0)
        # rounded F32R copies for the prolog GEMMs (DMA output may not feed
        # an FP32r matmult directly -- the producer must round)
        pr = {}
        for name in ["p_w", "d_w", "Wv0a", "Wv0b"]:
            t = sbw.tile([H, 128], F32R, tag=f"p_{name}")
            nc.vector.tensor_copy(t[:], w(wsb, name))
            pr[name] = t
        pfT_r = sbw.tile([H, NP], F32R, tag="pfT_r")
        nc.vector.tensor_copy(pfT_r[:], pfT[:])
        dfT_r = sbw.tile([H, ND], F32R, tag="dfT_r")
        nc.vector.tensor_copy(dfT_r[:], dfT[:])

        # ---- startup: projections, u_d/u_p, m_u, prot/drug proj ----
        ppT = sbw.tile([H, NP], F32, tag="ppT")
        dpT = sbw.tile([H, ND], F32, tag="dpT")
        u_p = sbw.tile([H, 1], F32, tag="u_p")
        u_d = sbw.tile([H, 1], F32, tag="u_d")
        mu_t = sbw.tile([H, 1], F32, tag="mu_t")
        acc = sbw.tile([H, NT], F32, tag="acc")

        with tc.tile_pool(name="ps0", bufs=2, space="PSUM") as ps0:
            # prot: feat = prelu(p_w.T @ pfT + p_b); masked + u_p
            ps_pf = ps0.tile([H, NP], F32, tag="ps0")
            nc.tensor.matmul(ps_pf[:], pr["p_w"][:], pfT_r[:], start=True, stop=True)
            pf_act = sbw.tile([H, NP], F32, tag="pf_act")
            nc.scalar.activation(pf_act[:], ps_pf[:], AF.Prelu,
                                 bias=bcol(bsb, "p_b"), scale=1.0, alpha=0.1)
            pf_m = sbw.tile([H, NP], F32R, tag="pf_m")
            nc.vector.scalar_tensor_tensor(pf_m[:], pf_act[:], 1.0, pmask[:],
                                           op0=OP.mult, op1=OP.mult,
                                           accum_out=u_p[:])
            # drug
            ps_df = ps0.tile([H, ND], F32, tag="ps0")
            nc.tensor.matmul(ps_df[:], pr["d_w"][:], dfT_r[:], start=True, stop=True)
            df_act = sbw.tile([H, ND], F32, tag="df_act")
            nc.scalar.activation(df_act[:], ps_df[:], AF.Prelu,
                                 bias=bcol(bsb, "d_b"), scale=1.0, alpha=0.1)
            df_m = sbw.tile([H, ND], F32R, tag="df_m")
            nc.vector.scalar_tensor_tensor(df_m[:], df_act[:], 1.0, dmask[:],
                                           op0=OP.mult, op1=OP.mult,
                                           accum_out=u_d[:])
            # pairwise projections (Wv0_b folded into prot side)
            ps_pp = ps0.tile([H, NP], F32, tag="ps0")
            nc.tensor.matmul(ps_pp[:], pr["Wv0a"][:], pf_m[:], start=True, stop=True)
            nc.scalar.activation(ppT[:], ps_pp[:], AF.Identity,
                                 bias=bcol(bsb, "Wv0_b"), scale=1.0)
            ps_dp = ps0.tile([H, ND], F32, tag="ps0")
            nc.tensor.matmul(ps_dp[:], pr["Wv0b"][:], df_m[:], start=True, stop=True)
            nc.scalar.copy(dpT[:], ps_dp[:])
            # m_u = prelu(Wu.T @ [u_d;u_p] + Wu_b, .01)
            ps_mu = ps0.tile([H, 1], F32, tag="ps0s")
            nc.tensor.matmul(ps_mu[:], w(wsb, "Wu_0"), u_d[:], start=True, stop=False)
            nc.tensor.matmul(ps_mu[:], w(wsb, "Wu_1"), u_p[:], start=False, stop=True)
            nc.scalar.activation(mu_t[:], ps_mu[:], AF.Prelu,
                                 bias=bcol(bsb, "Wu_b"), scale=1.0, alpha=0.01)

        # ---- main loop over macro tiles (software-pipelined emission) ----
        # Stages per tile t:
        #   S0: construct m0        S1/S2/S3: depth 0/1/2        S4: W1a+relu
        # Emitted deepest-stage-first per step so each engine's instruction
        # stream interleaves stages of staggered tiles (no cross-tile stalls).
        PSA = int(_os.environ.get("K_PSA", "3"))
        PST = int(_os.environ.get("K_PST", "3"))
        PSW = int(_os.environ.get("K_PSW", "1"))
        PAIR = int(_os.environ.get("K_PAIR", "1"))
        with tc.tile_pool(name="sbm", bufs=16 if not F32R_MODE else 12) as sbm, \
             tc.tile_pool(name="sbg", bufs=8 if not F32R_MODE else 6) as sbg, \
             tc.tile_pool(name="pst", bufs=PST, space="PSUM") as pst, \
             tc.tile_pool(name="psa", bufs=PSA, space="PSUM") as psa, \
             tc.tile_pool(name="psw", bufs=PSW, space="PSUM") as psw:
            NTL = NT
            # test.py wraps the main loop in an on-device For_i for timing
            reps = int(_os.environ.get("K_REPS", "0"))
            m0set, m1set, m2set, g_set = {}, {}, {}, {}
            pa_set, pt_set, al_set, pwpair = {}, {}, {}, {}

            # Legal-ISA schedule (GPSIMD cannot touch PSUM; every other op
            # reads at most ONE PSUM operand, so each depth needs an
            # alpha-mover (ACT/DVE) plus a g = (pt+b)*al stt on DVE):
            #   al_i = pa_i + c_i   (PSUM->SBUF, ACT Identity-bias or DVE ts)
            #   g_i  = (pt_i + b_i) * al_i    [DVE stt, pt in PSUM]
            #   m   += g                      [Pool tt, SBUF bf16]
            # vrep_2/c2 are pre-doubled so g2 = 2*g2_true and the W1a stage
            # is W1a@m2 + W1a@g2 accumulated in PSUM (w1in never built).
            # ACT carries ~2.14 of the 3 movers (s0 + relu fill the rest);
            # the remainder runs on DVE, mixed by tile parity.

            AL2ACT = int(_os.environ.get("K_AL2ACT", "7"))  # al2 on ACT every Nth
            AL1DVE = int(_os.environ.get("K_AL1DVE", "0"))  # al1 on DVE every Nth
            ADDDVE = int(_os.environ.get("K_ADDDVE", "0"))  # m-adds on DVE every Nth

            def d_mm(i, t, m):
                pa = psa.tile([H, FD], F32, tag="pa")
                pt = pst.tile([H, FD], F32, tag="pt")
                nc.tensor.matmul(pa[:], wr[f"vrep_{i}"], m[:],
                                 start=True, stop=True)
                nc.tensor.matmul(pt[:], wr[f"Wv_{i}"], m[:],
                                 start=True, stop=True)
                pa_set[(i, t)], pt_set[(i, t)] = pa, pt

            def d_al(i, t, on_act):
                pa = pa_set.pop((i, t))
                al = sbg.tile([H, FD], BF16, tag=f"al{i}")
                cname = ("c0", "c1", "c2x2")[i]
                if on_act:
                    nc.scalar.activation(al[:], pa[:], AF.Identity,
                                         bias=bcol(bsb, cname), scale=1.0)
                else:
                    nc.vector.tensor_scalar_add(al[:], pa[:],
                                                float((c0_v, c1_v, c2x2_v)[i]))
                al_set[(i, t)] = al

            def d_g(i, t):
                pt, al = pt_set.pop((i, t)), al_set.pop((i, t))
                g = sbg.tile([H, FD], MD, tag=f"g{i}")
                nc.vector.scalar_tensor_tensor(g[:], pt[:],
                                               bcol(bsb, f"Wv_b{i}"), al[:],
                                               op0=OP.add, op1=OP.mult)
                g_set[(i, t)] = g

            def d_add(i, t, msrc, mdst_set):
                g = g_set.pop((i, t))
                m2 = sbm.tile([H, FD], MD, tag=f"m{i + 1}")
                ma, ga = msrc.pop(t)[:], g[:]
                if F32R_MODE:
                    ma, ga = ma.bitcast(F32), ga.bitcast(F32)
                if ADDDVE and t % ADDDVE == 0:
                    nc.vector.tensor_tensor(m2[:], ma, ga, op=OP.add)
                else:
                    nc.gpsimd.tensor_tensor(m2[:], ma, ga, op=OP.add)
                mdst_set[t] = m2

            def s0(t):
                m0 = sbm.tile([H, FD], MD, tag="m0")
                nc.scalar.activation(m0[:], ppT[:], AF.Prelu,
                                     bias=dpT[:, t:t + 1], scale=1.0, alpha=0.1)
                m0set[t] = m0

            def s4_mm(t):
                m2, g2 = m2set.pop(t), g_set.pop((2, t))
                if PAIR:
                    if t % 2 == 0:
                        pwpair[0] = psw.tile([H, 2 * FD], F32, tag="pw",
                                             name="pwp")
                    pw = pwpair[0]
                    half = slice((t % 2) * FD, (t % 2 + 1) * FD)
                else:
                    pwpair[0] = psw.tile([H, FD], F32, tag="pw", name="pwp")
                    pw = pwpair[0]
                    half = slice(0, FD)
                nc.tensor.matmul(pw[:, half], wr["W1a"], m2[:],
                                 start=True, stop=False)
                nc.tensor.matmul(pw[:, half], wr["W1a"], g2[:],
                                 start=False, stop=True)

            def s4_relu(t):
                if PAIR:
                    if t % 2 == 1:
                        pw = pwpair[0]
                        scr = sbg.tile([H, 2 * FD], BF16, tag="scr")
                        nc.scalar.activation(scr[:], pw[:], AF.Relu,
                                             bias=bcol(bsb, "W1a_b"), scale=1.0,
                                             accum_out=acc[:, t // 2:t // 2 + 1])
                else:
                    pw = pwpair[0]
                    scr = sbg.tile([H, FD], BF16, tag="scr")
                    nc.scalar.activation(scr[:], pw[:], AF.Relu,
                                         bias=bcol(bsb, "W1a_b"), scale=1.0,
                                         accum_out=acc[:, t:t + 1])

            LAG = int(_os.environ.get('K_LAG', '3'))

            def main_loop():
                for step in range(NTL + 4 * LAG):
                    t1 = step - LAG          # depth-0 tile
                    t2 = step - 2 * LAG      # depth-1 tile
                    t3 = step - 3 * LAG      # depth-2 tile
                    t4 = step - 4 * LAG      # W1a tile
                    live1 = 0 <= t1 < NTL
                    live2 = 0 <= t2 < NTL
                    live3 = 0 <= t3 < NTL
                    live4 = 0 <= t4 < NTL
                    al2_act = AL2ACT and t3 % AL2ACT == 0
                    al1_dve = AL1DVE and t2 % AL1DVE == 0
                    if live1:
                        d_mm(0, t1, m0set[t1])
                        d_al(0, t1, True)       # ACT
                        d_g(0, t1)              # DVE
                    if live3:
                        d_mm(2, t3, m2set[t3])
                        d_al(2, t3, al2_act)    # DVE (ACT every Nth)
                    if step < NTL:
                        s0(step)
                    if live3:
                        d_g(2, t3)              # DVE
                    if live1:
                        d_add(0, t1, m0set, m1set)   # Pool
                    if live2:
                        d_mm(1, t2, m1set[t2])
                        d_al(1, t2, not al1_dve)     # ACT (DVE every Nth)
                        d_g(1, t2)              # DVE
                        d_add(1, t2, m1set, m2set)   # Pool
                    if live4:
                        s4_mm(t4)
                        s4_relu(t4)

            if reps:
                with tc.For_i(0, reps, 1,
                              hint_engines=(mybir.EngineType.PE,
                                            mybir.EngineType.Activation,
                                            mybir.EngineType.DVE,
                                            mybir.EngineType.Pool,
                                            mybir.EngineType.SP)):
                    main_loop()
            else:
                main_loop()


        # ---- tail MLP (tiny) ----
        with tc.tile_pool(name="pse", bufs=2, space="PSUM") as pse, \
             tc.tile_pool(name="sbe", bufs=1) as sbe:
            s_t = sbe.tile([H, 1], F32, tag="s")
            nacc = NT // 2 if int(_os.environ.get("K_PAIR", "1")) else NT
            nc.vector.tensor_reduce(s_t[:], acc[:, 0:nacc],
                                    axis=mybir.AxisListType.X, op=OP.add)
            p1 = pse.tile([H, 1], F32, tag="pse")
            nc.tensor.matmul(p1[:], w(wsb, "W1b"), s_t[:], start=True, stop=True)
            t1 = sbe.tile([H, 1], F32, tag="t1")
            nc.scalar.activation(t1[:], p1[:], AF.Identity,
                                 bias=bcol(bsb, "W1b_bN"), scale=1.0)
            p2a = pse.tile([H, 2], F32, tag="pse")
            nc.tensor.matmul(p2a[:, 0:1], w(wsb, "W2a_0"), t1[:], start=True, stop=True)
            nc.tensor.matmul(p2a[:, 1:2], w(wsb, "W2a_1"), t1[:], start=True, stop=True)
            t2a = sbe.tile([H, 2], F32, tag="t2a")
            nc.scalar.activation(t2a[:, 0:1], p2a[:, 0:1], AF.Prelu,
                                 bias=bcol(bsb, "W2a_b0"), scale=1.0, alpha=0.1)
            nc.scalar.activation(t2a[:, 1:2], p2a[:, 1:2], AF.Prelu,
                                 bias=bcol(bsb, "W2a_b1"), scale=1.0, alpha=0.1)
            p2 = pse.tile([H, 1], F32, tag="pse")
            nc.tensor.matmul(p2[:], w(wsb, "W2b_0"), t2a[:, 0:1], start=True, stop=False)
            nc.tensor.matmul(p2[:], w(wsb, "W2b_1"), t2a[:, 1:2], start=False, stop=True)
            t2 = sbe.tile([H, 1], F32, tag="t2")
            nc.scalar.activation(t2[:], p2[:], AF.Identity,
                                 bias=bcol(bsb, "W2b_b"), scale=1.0)
            p3 = pse.tile([H, 1], F32, tag="pse")
            nc.tensor.matmul(p3[:], w(wsb, "W3"), t2[:], start=True, stop=True)
            t3 = sbe.tile([H, 1], F32, tag="t3")
            nc.scalar.activation(t3[:], p3[:], AF.Prelu,
                                 bias=bcol(bsb, "W3_b"), scale=1.0, alpha=0.1)
            p4 = pse.tile([H, 1], F32, tag="pse")
            nc.tensor.matmul(p4[:], w(wsb, "Wu1a_0"), mu_t[:], start=True, stop=False)
            nc.tensor.matmul(p4[:], w(wsb, "Wu1a_1"), t3[:], start=False, stop=True)
            t4 = sbe.tile([H, 1], F32, tag="t4")
            nc.scalar.activation(t4[:], p4[:], AF.Prelu,
                                 bias=bcol(bsb, "Wu1a_b"), scale=1.0, alpha=0.1)
            p5 = pse.tile([H, 1], F32, tag="pse")
            nc.tensor.matmul(p5[:], w(wsb, "Wu1b"), t4[:], start=True, stop=True)
            t5 = sbe.tile([H, 1], F32, tag="t5")
            nc.scalar.activation(t5[:], p5[:], AF.Identity,
                                 bias=bcol(bsb, "Wu1b_b"), scale=1.0)
            p6 = pse.tile([1, 1], F32, tag="pse")
            nc.tensor.matmul(p6[:], bcol(bsb, "W5_w"), t5[:], start=True, stop=True)
            o_sb = sbe.tile([1, 1], F32, tag="o")
            nc.scalar.activation(o_sb[:], p6[:], AF.Identity,
                                 bias=bsb[0:1, BIDX["W5_b"]:BIDX["W5_b"] + 1],
                                 scale=1.0)
            nc.sync.dma_start(out=out_d[:], in_=o_sb[:])

    nc.compile()
    return nc


_NC = None


def _get_nc():
    global _NC
    if _NC is None or _NC[0] != _CONSTS:
        _NC = (_CONSTS, build_program())
    return _NC[1]


def _host_prep(inputs):
    """Build per-core in_maps from full inputs (weight transforms on host)."""
    f = {k: np.asarray(v, dtype=np.float32) for k, v in inputs.items()}

    # v_i = Wv_w[i] @ att_w[i],  c_i = att_w[i].Wv_b[i] + att_b[i]
    Wv_w, att_w = f["Wv_w"], f["att_w"]
    Wv_b, att_b = f["Wv_b"], f["att_b"]
    vs = [Wv_w[i] @ att_w[i] for i in range(DEPTH)]
    cs = [float(att_w[i] @ Wv_b[i] + att_b[i]) for i in range(DEPTH)]
    global _CONSTS
    _CONSTS = (cs[0], cs[1], 2.0 * cs[2])

    wmcols = {
        "Wv_0": Wv_w[0], "Wv_1": Wv_w[1], "Wv_2": Wv_w[2],
        "vrep_0": np.repeat(vs[0][:, None], 128, 1),
        "vrep_1": np.repeat(vs[1][:, None], 128, 1),
        "vrep_2": np.repeat(2.0 * vs[2][:, None], 128, 1),
        "W1a": f["W1a_w"],
    }
    import ml_dtypes
    wmain = np.concatenate([np.ascontiguousarray(wmcols[n])
                            for n in WM_NAMES], axis=1)
    if os.environ.get("K_DT", "bf16") != "f32r":
        wmain = wmain.astype(ml_dtypes.bfloat16)
    wcols = {
        "p_w": f["p_w"], "d_w": f["d_w"],
        "Wv0a": f["Wv0_w"][:H], "Wv0b": f["Wv0_w"][H:],
        "W1b": f["W1b_w"],
        "W2a_0": f["W2a_w"][:, :H], "W2a_1": f["W2a_w"][:, H:],
        "W2b_0": f["W2b_w"][:H], "W2b_1": f["W2b_w"][H:],
        "W3": f["W3_w"],
        "Wu_0": f["Wu_w"][:H], "Wu_1": f["Wu_w"][H:],
        "Wu1a_0": f["Wu1a_w"][:H], "Wu1a_1": f["Wu1a_w"][H:],
        "Wu1b": f["Wu1b_w"],
    }
    wmat = np.concatenate([np.ascontiguousarray(wcols[n]) for n in W_NAMES],
                          axis=1)

    def bc(v):
        v = np.asarray(v, dtype=np.float32).reshape(-1)
        if v.size == 1:
            return np.full((H,), float(v[0]), dtype=np.float32)
        assert v.size == H
        return v

    bvals = {
        "p_b": f["p_b"], "d_b": f["d_b"], "Wv0_b": f["Wv0_b"],
        "Wv_b0": Wv_b[0], "Wv_b1": Wv_b[1], "Wv_b2": Wv_b[2],
        "c0": cs[0], "c1": cs[1], "c2x2": 2.0 * cs[2],
        "Wu_b": f["Wu_b"], "W1a_b": f["W1a_b"],
        "W1b_bN": f["W1b_b"] * np.float32(N),
        "W2a_b0": f["W2a_b"][:H], "W2a_b1": f["W2a_b"][H:],
        "W2b_b": f["W2b_b"], "W3_b": f["W3_b"],
        "Wu1a_b": f["Wu1a_b"], "Wu1b_b": f["Wu1b_b"],
        "W5_b": f["W5_b"], "W5_w": f["W5_w"][:, 0],
    }
    bmat = np.stack([bc(bvals[n]) for n in B_NAMES], axis=1)
    bmat = np.ascontiguousarray(bmat, dtype=np.float32)

    pf, df = f["protein_features"], f["drug_features"]
    pm, dm = f["pu_mask"], f["du_mask"]
    in_maps = []
    for b in range(B):
        in_maps.append({
            "pfT": np.ascontiguousarray(pf[b].T),
            "dfT": np.ascontiguousarray(df[b].T),
            "pmask": np.ascontiguousarray(
                np.broadcast_to(pm[b][None, :], (H, NP))),
            "dmask": np.ascontiguousarray(
                np.broadcast_to(dm[b][None, :], (H, ND))),
            "wmat": wmat,
            "wmain": wmain,
            "bmat": bmat,
        })
    return in_maps


def kernel(**inputs) -> np.ndarray:
    in_maps = _host_prep(inputs)
    nc = _get_nc()
    res = run_bass_kernel_spmd(nc, in_maps, list(range(B)))
    out = np.concatenate([res.results[b]["out"] for b in range(B)], axis=0)
    return out.astype(np.float32).reshape(B, 1)
